# revision 12
# baseline (speedup 1.0000x reference)
"""Trainium2 Bass kernel for the semantic-weighted contrastive loss.

Problem (full shapes): audio [8192,1024] f32, text [4096,1024] f32,
semantic_weights [8192,4096] f32, pos_idx [8192] i32 -> scalar f32 loss.

Strategy: data-parallel over B across 8 NeuronCores (1024 rows/core).
All O(B*D)/O(C*D) prep runs on the host in f32 (L2-normalize, positive-pair
logits, transposes, down-casts); the device does only the O(B*C*D) matmul
and the O(B*C) exp/weighted-reduce:

  host:  an = normalize(audio); tn = normalize(text)
         pos_logit[b] = (an[b] . tn[pos_b]) / T            (f32)
         aT = (an*16).T  as fp8  [128, KT=8, 1024]         (k-major tiles)
         tT = (tn*16).T  as fp8  [128, CC=8, KT=8, 512]
         semc = (1-sem)  as bf16 [128, NBT=8, 4096]
  core:  for bt, cc:  psum[128,512] = sum_k aT.T @ tT      (fp8 DoubleRow,
             4 matmuls of K=256 each, f32 accumulate)
         ex = exp(psum * (1/T/256))  -> bf16               (ACT)
         W[b] = sum_c ex * semc                            (DVE fused
             tensor_tensor_reduce, f32 accum)
  host:  denom = W + exp(pos_logit)*sem_pos                (pos-correction:
             W includes the c=pos term exp(lpos)*(1-sem_pos); adding
             exp(pos)*sem_pos replaces it with exp(pos) up to fp8 noise)
         loss = mean(-pos_logit + log(denom))

fp8 e4m3 logits carry ~0.02 absolute noise; emulated end-to-end rel err
vs the f32 reference is ~2e-5 (gate: 2e-2).
"""

import sys

for _p in ("/opt/trn_rl_repo", "/root/.axon_site/_ro/trn_rl_repo"):
    if _p not in sys.path:
        sys.path.append(_p)

import numpy as np
import ml_dtypes

import concourse.bass as bass
import concourse.mybir as mybir
import concourse.tile as tile
from concourse.bass_utils import run_bass_kernel_spmd

F32 = mybir.dt.float32
BF16 = mybir.dt.bfloat16
F8 = mybir.dt.float8e4
AF = mybir.ActivationFunctionType
ALU = mybir.AluOpType
PMODE = mybir.MatmulPerfMode

B, C, D = 8192, 4096, 1024
TEMPERATURE = 0.07
INV_T = 1.0 / TEMPERATURE
NCORES = 8
BL = B // NCORES   # 1024 rows per core
P = 128
KT = D // P        # 8 k-tiles of 128
NKP = KT // 2      # 4 DoubleRow pairs (K=256 each)
NCHUNK = 512
NCC = C // NCHUNK  # 8 c-chunks
NBT = BL // P      # 8 b-tiles per core
SF = 16.0          # fp8 pre-scale; undone by ACT_SCALE
ACT_SCALE = INV_T / (SF * SF)

NP_F8 = ml_dtypes.float8_e4m3
NP_BF16 = ml_dtypes.bfloat16


def _build_nc() -> bass.Bass:
    nc = bass.Bass()
    at = nc.declare_dram_parameter("at", [P, NBT, KT, P], F8, isOutput=False)
    tt = nc.declare_dram_parameter("tt", [P, NCC, KT, NCHUNK], F8, isOutput=False)
    semc = nc.declare_dram_parameter(
        "semc", [P, NCC, NBT, NCHUNK], F8, isOutput=False
    )
    wout = nc.declare_dram_parameter("wsum", [P, NBT], F32, isOutput=True)

    # The container's walrus (May-2026 b16 fork) rejects the ANT
    # EVENT_SEMAPHORE_RANGE_CLEAR InstISA that Tile's exit path emits
    # ("ISA wrong length"). Skip emitting it; the NEFF is re-loaded per
    # invocation here, so semaphores start from their load-time state.
    orig_sem_clear = type(nc.gpsimd).sem_clear
    type(nc.gpsimd).sem_clear = lambda self, sem: None
    try:
        with tile.TileContext(nc) as tc:
            _body(tc, at, tt, semc, wout)
    finally:
        type(nc.gpsimd).sem_clear = orig_sem_clear
    # Populate .instr bytes for extended-ISA instructions (tensor_tensor_reduce
    # et al). Bacc.compile() runs this; the raw-Bass path we use does not, and
    # walrus fails on empty .instr with "ISA wrong length".
    mybir.codegen_inst_isa_subclasses(nc)
    _split_waits(nc)
    nc.finalize()
    return nc


def _split_waits(nc):
    """The container's walrus allows only ONE sync-wait per TPB instruction
    (it errors with "Too many sync wait commands" otherwise). Hoist extra
    waits into standalone same-engine EventSemaphore wait instructions,
    inserted immediately before the owner. Engines execute their stream in
    order, so blocking behavior is identical."""
    n_new = 0
    for fn in nc.m.functions:
        for bb in fn.blocks:
            new_list = []
            for inst in bb.instructions:
                si = getattr(inst, "sync_info", None)
                if si and si.on_wait and len(si.on_wait) > 1:
                    extra, keep = si.on_wait[:-1], si.on_wait[-1:]
                    for w in extra:
                        n_new += 1
                        wi = mybir.InstEventSemaphore(
                            name=f"{inst.name}_w{n_new}",
                            engine=inst.engine,
                            ins=[],
                            outs=[],
                            sync_info=mybir.SyncInfo(on_wait=[w], on_update=[]),
                        )
                        nc.inst_map[wi.name] = wi
                        new_list.append(wi)
                    si.on_wait = keep
                new_list.append(inst)
            bb.instructions[:] = new_list


def _body(tc, at, tt, semc, wout):
    nc = tc.nc
    from contextlib import ExitStack

    with ExitStack() as ctx:
        res = ctx.enter_context(tc.tile_pool(name="res", bufs=1))
        expp = ctx.enter_context(tc.tile_pool(name="expp", bufs=3))
        dpool = ctx.enter_context(tc.tile_pool(name="dump", bufs=2))
        pm = ctx.enter_context(tc.tile_pool(name="pmm", bufs=8, space="PSUM"))

        aT = res.tile([P, NBT, KT, P], F8, tag="aT")
        tT = res.tile([P, NCC, KT, NCHUNK], F8, tag="tT")
        sc = res.tile([P, NCC, NBT, NCHUNK], F8, tag="sc")
        es = res.tile([P, NBT, NCC], F32, tag="es")
        ws = res.tile([P, NBT], F32, tag="ws")
        warm = res.tile([P, NCHUNK], BF16, tag="warm")

        # HAM warm-up: PE sits idle ~4us waiting for the first input DMAs;
        # dummy matmuls on a zeroed tile keep it busy so the clock gate is at
        # 8/8 (2.4 GHz) when the real matmuls start.
        nc.vector.memset(warm[:], 0)
        for w in range(11):
            pw = pm.tile([P, NCHUNK], F32, tag="ps", name=f"warm{w}")
            nc.tensor.matmul(
                pw[:], lhsT=warm[:, 0:P], rhs=warm[:], start=True, stop=True
            )

        # input DMAs, split across all three DMA-capable queues (each queue
        # injects ~115GB/s early on): sync feeds the PE-critical tT chunks,
        # scalar feeds the aT b-tiles, gpsimd feeds the semc slabs; each
        # queue's order matches consumption order.
        for cc in range(NCC):
            nc.sync.dma_start(tT[:, cc, :, :], tt[:, cc, :, :])
        for bt in range(NBT):
            nc.scalar.dma_start(aT[:, bt, :, :], at[:, bt, :, :])
        for cc in range(NCC):
            nc.gpsimd.dma_start(sc[:, cc, :, :], semc[:, cc, :, :])

        # c-chunk-outer: each strip needs just one 512KB tT chunk, so the
        # DMA stream stays ahead of the PE and it never goes idle/cold.
        for cc in range(NCC):
            for bt in range(NBT):
                ps = pm.tile([P, NCHUNK], F32, tag="ps", name=f"ps{cc}_{bt}")
                for kp in range(NKP):
                    nc.tensor.matmul(
                        ps[:],
                        lhsT=aT[:, bt, 2 * kp : 2 * kp + 2, :],
                        rhs=tT[:, cc, 2 * kp : 2 * kp + 2, :],
                        start=(kp == 0),
                        stop=(kp == NKP - 1),
                        perf_mode=PMODE.DoubleRow,
                    )
                ex = expp.tile([P, NCHUNK], BF16, tag="ex")
                nc.scalar.activation(ex[:], ps[:], AF.Exp, scale=ACT_SCALE)
                # fused W chunk: out = (ex * 1.0) * (1-sem), accum = sum
                # (tensor_tensor_reduce is rejected by this runtime;
                # scalar_tensor_tensor is standard BIR and works)
                dmp = dpool.tile([P, NCHUNK], BF16, tag="dmp")
                nc.vector.scalar_tensor_tensor(
                    out=dmp[:],
                    in0=ex[:],
                    scalar=1.0,
                    in1=sc[:, cc, bt, :],
                    op0=ALU.mult,
                    op1=ALU.mult,
                    accum_out=es[:, bt, cc : cc + 1],
                )
        nc.vector.reduce_sum(ws[:], es[:], axis=mybir.AxisListType.X)
        nc.scalar.dma_start(wout[:], ws[:])


_NC_CACHE = None


def _get_nc() -> bass.Bass:
    global _NC_CACHE
    if _NC_CACHE is None:
        _NC_CACHE = _build_nc()
    return _NC_CACHE


def _host_prep(audio_embeddings, text_embeddings, semantic_weights, pos_idx):
    """f32 host prep: normalize, positive logits, device operand packing."""
    a = np.asarray(audio_embeddings, dtype=np.float32)
    t = np.asarray(text_embeddings, dtype=np.float32)
    sem = np.asarray(semantic_weights, dtype=np.float32)
    pos = np.asarray(pos_idx, dtype=np.int32)

    an = a / np.maximum(np.linalg.norm(a, axis=1, keepdims=True), 1e-12)
    tn = t / np.maximum(np.linalg.norm(t, axis=1, keepdims=True), 1e-12)
    pos_log = np.einsum("bd,bd->b", an, tn[pos]).astype(np.float32) * np.float32(
        INV_T
    )
    sem_pos = sem[np.arange(B), pos]

    # tT: [D, C] -> [P, NCC, KT, NCHUNK] with k = kt*128 + p, c = cc*512 + j
    t8 = (tn * SF).astype(NP_F8).T.reshape(KT, P, NCC, NCHUNK)
    tt_host = np.ascontiguousarray(t8.transpose(1, 2, 0, 3))

    in_maps = []
    for k in range(NCORES):
        sl = slice(k * BL, (k + 1) * BL)
        # at[p, bt, kt, j] = an[bt*128+j, kt*128+p] * SF
        a8 = (an[sl] * SF).astype(NP_F8).T.reshape(KT, P, NBT, P)
        at_host = np.ascontiguousarray(a8.transpose(1, 2, 0, 3))
        # semc[p, cc, bt, j] = 1 - sem[bt*128+p, cc*512+j]
        s16 = (1.0 - sem[sl]).astype(NP_F8).reshape(NBT, P, NCC, NCHUNK)
        semc_host = np.ascontiguousarray(s16.transpose(1, 2, 0, 3))
        in_maps.append({"at": at_host, "tt": tt_host, "semc": semc_host})
    return in_maps, pos_log, sem_pos


def run_sharded(inputs: dict, trace: bool = False):
    """Run on the 8 NeuronCores; returns (loss_scalar, BassKernelResults)."""
    nc = _get_nc()
    in_maps, pos_log, sem_pos = _host_prep(**inputs)
    res = run_bass_kernel_spmd(
        nc,
        in_maps,
        list(range(NCORES)),
        trace=trace,
        trace_cores=[0] if trace else None,
    )
    # wsum[p, bt] = W[bt*128 + p] for the core's shard
    W = np.concatenate([r["wsum"].T.reshape(BL) for r in res.results])
    den = W + np.exp(pos_log) * sem_pos
    loss = -pos_log + np.log(den)
    val = np.float32(loss.mean(dtype=np.float64))
    return val, res


def kernel(**inputs) -> np.ndarray:
    val, _ = run_sharded(inputs, trace=False)
    return np.asarray(val, dtype=np.float32)


# revision 13
# speedup vs baseline: 1.1973x; 1.1973x over previous
"""Trainium2 Bass kernel for the semantic-weighted contrastive loss.

Problem (full shapes): audio [8192,1024] f32, text [4096,1024] f32,
semantic_weights [8192,4096] f32, pos_idx [8192] i32 -> scalar f32 loss.

Strategy: data-parallel over B across 8 NeuronCores (1024 rows/core).
All O(B*D)/O(C*D) prep runs on the host in f32 (L2-normalize, positive-pair
logits, transposes, down-casts); the device does only the O(B*C*D) matmul
and the O(B*C) exp/weighted-reduce:

  host:  an = normalize(audio); tn = normalize(text)
         pos_logit[b] = (an[b] . tn[pos_b]) / T            (f32)
         aT = (an*16).T  as fp8  [128, KT=8, 1024]         (k-major tiles)
         tT = (tn*16).T  as fp8  [128, CC=8, KT=8, 512]
         semc = (1-sem)  as bf16 [128, NBT=8, 4096]
  core:  for bt, cc:  psum[128,512] = sum_k aT.T @ tT      (fp8 DoubleRow,
             4 matmuls of K=256 each, f32 accumulate)
         ex = exp(psum * (1/T/256))  -> bf16               (ACT)
         W[b] = sum_c ex * semc                            (DVE fused
             tensor_tensor_reduce, f32 accum)
  host:  denom = W + exp(pos_logit)*sem_pos                (pos-correction:
             W includes the c=pos term exp(lpos)*(1-sem_pos); adding
             exp(pos)*sem_pos replaces it with exp(pos) up to fp8 noise)
         loss = mean(-pos_logit + log(denom))

fp8 e4m3 logits carry ~0.02 absolute noise; emulated end-to-end rel err
vs the f32 reference is ~2e-5 (gate: 2e-2).
"""

import sys

for _p in ("/opt/trn_rl_repo", "/root/.axon_site/_ro/trn_rl_repo"):
    if _p not in sys.path:
        sys.path.append(_p)

import numpy as np
import ml_dtypes

import concourse.bass as bass
import concourse.mybir as mybir
import concourse.tile as tile
from concourse.bass_utils import run_bass_kernel_spmd

F32 = mybir.dt.float32
BF16 = mybir.dt.bfloat16
F8 = mybir.dt.float8e4
AF = mybir.ActivationFunctionType
ALU = mybir.AluOpType
PMODE = mybir.MatmulPerfMode

B, C, D = 8192, 4096, 1024
TEMPERATURE = 0.07
INV_T = 1.0 / TEMPERATURE
NCORES = 8
BL = B // NCORES   # 1024 rows per core
P = 128
KT = D // P        # 8 k-tiles of 128
NKP = KT // 2      # 4 DoubleRow pairs (K=256 each)
NCHUNK = 512
NCC = C // NCHUNK  # 8 c-chunks
NBT = BL // P      # 8 b-tiles per core
SF = 16.0          # fp8 pre-scale; undone by ACT_SCALE
ACT_SCALE = INV_T / (SF * SF)

NP_F8 = ml_dtypes.float8_e4m3
NP_BF16 = ml_dtypes.bfloat16


def _build_nc() -> bass.Bass:
    nc = bass.Bass()
    at = nc.declare_dram_parameter("at", [P, NBT, KT, P], F8, isOutput=False)
    tt = nc.declare_dram_parameter("tt", [P, NCC, KT, NCHUNK], F8, isOutput=False)
    semc = nc.declare_dram_parameter(
        "semc", [P, NCC, NBT, NCHUNK], F8, isOutput=False
    )
    wout = nc.declare_dram_parameter("wsum", [P, NBT], F32, isOutput=True)

    # The container's walrus (May-2026 b16 fork) rejects the ANT
    # EVENT_SEMAPHORE_RANGE_CLEAR InstISA that Tile's exit path emits
    # ("ISA wrong length"). Skip emitting it; the NEFF is re-loaded per
    # invocation here, so semaphores start from their load-time state.
    orig_sem_clear = type(nc.gpsimd).sem_clear
    type(nc.gpsimd).sem_clear = lambda self, sem: None
    try:
        with tile.TileContext(nc) as tc:
            _body(tc, at, tt, semc, wout)
    finally:
        type(nc.gpsimd).sem_clear = orig_sem_clear
    # Populate .instr bytes for extended-ISA instructions (tensor_tensor_reduce
    # et al). Bacc.compile() runs this; the raw-Bass path we use does not, and
    # walrus fails on empty .instr with "ISA wrong length".
    mybir.codegen_inst_isa_subclasses(nc)
    _split_waits(nc)
    nc.finalize()
    return nc


def _split_waits(nc):
    """The container's walrus allows only ONE sync-wait per TPB instruction
    (it errors with "Too many sync wait commands" otherwise). Hoist extra
    waits into standalone same-engine EventSemaphore wait instructions,
    inserted immediately before the owner. Engines execute their stream in
    order, so blocking behavior is identical."""
    n_new = 0
    for fn in nc.m.functions:
        for bb in fn.blocks:
            new_list = []
            for inst in bb.instructions:
                si = getattr(inst, "sync_info", None)
                if si and si.on_wait and len(si.on_wait) > 1:
                    extra, keep = si.on_wait[:-1], si.on_wait[-1:]
                    for w in extra:
                        n_new += 1
                        wi = mybir.InstEventSemaphore(
                            name=f"{inst.name}_w{n_new}",
                            engine=inst.engine,
                            ins=[],
                            outs=[],
                            sync_info=mybir.SyncInfo(on_wait=[w], on_update=[]),
                        )
                        nc.inst_map[wi.name] = wi
                        new_list.append(wi)
                    si.on_wait = keep
                new_list.append(inst)
            bb.instructions[:] = new_list


def _body(tc, at, tt, semc, wout):
    nc = tc.nc
    from contextlib import ExitStack

    with ExitStack() as ctx:
        res = ctx.enter_context(tc.tile_pool(name="res", bufs=1))
        expp = ctx.enter_context(tc.tile_pool(name="expp", bufs=3))
        dpool = ctx.enter_context(tc.tile_pool(name="dump", bufs=2))
        pm = ctx.enter_context(tc.tile_pool(name="pmm", bufs=8, space="PSUM"))

        aT = res.tile([P, NBT, KT, P], F8, tag="aT")
        tT = res.tile([P, NCC, KT, NCHUNK], F8, tag="tT")
        sc = res.tile([P, NCC, NBT, NCHUNK], F8, tag="sc")
        es = res.tile([P, NBT, NCC], F32, tag="es")
        ws = res.tile([P, NBT], F32, tag="ws")
        warm = res.tile([P, NCHUNK], BF16, tag="warm")

        # HAM warm-up: PE sits idle ~4us waiting for the first input DMAs;
        # dummy matmuls on a zeroed tile keep it busy so the clock gate is at
        # 8/8 (2.4 GHz) when the real matmuls start.
        nc.vector.memset(warm[:], 0)
        for w in range(11):
            pw = pm.tile([P, NCHUNK], F32, tag="ps", name=f"warm{w}")
            nc.tensor.matmul(
                pw[:], lhsT=warm[:, 0:P], rhs=warm[:], start=True, stop=True
            )

        # input DMAs, split across two issuing queues (a third queue causes
        # ring stalls): sync feeds the PE-critical tT chunks, gpsimd feeds
        # the aT b-tiles then semc slabs; orders match consumption order.
        for cc in range(NCC):
            nc.sync.dma_start(tT[:, cc, :, :], tt[:, cc, :, :])
        for bt in range(NBT):
            nc.gpsimd.dma_start(aT[:, bt, :, :], at[:, bt, :, :])
        for cc in range(NCC):
            nc.gpsimd.dma_start(sc[:, cc, :, :], semc[:, cc, :, :])

        # c-chunk-outer: each strip needs just one 512KB tT chunk, so the
        # DMA stream stays ahead of the PE and it never goes idle/cold.
        for cc in range(NCC):
            for bt in range(NBT):
                ps = pm.tile([P, NCHUNK], F32, tag="ps", name=f"ps{cc}_{bt}")
                for kp in range(NKP):
                    nc.tensor.matmul(
                        ps[:],
                        lhsT=aT[:, bt, 2 * kp : 2 * kp + 2, :],
                        rhs=tT[:, cc, 2 * kp : 2 * kp + 2, :],
                        start=(kp == 0),
                        stop=(kp == NKP - 1),
                        perf_mode=PMODE.DoubleRow,
                    )
                ex = expp.tile([P, NCHUNK], BF16, tag="ex")
                nc.scalar.activation(ex[:], ps[:], AF.Exp, scale=ACT_SCALE)
                # fused W chunk: out = (ex * 1.0) * (1-sem), accum = sum
                # (tensor_tensor_reduce is rejected by this runtime;
                # scalar_tensor_tensor is standard BIR and works)
                dmp = dpool.tile([P, NCHUNK], BF16, tag="dmp")
                nc.vector.scalar_tensor_tensor(
                    out=dmp[:],
                    in0=ex[:],
                    scalar=1.0,
                    in1=sc[:, cc, bt, :],
                    op0=ALU.mult,
                    op1=ALU.mult,
                    accum_out=es[:, bt, cc : cc + 1],
                )
        nc.vector.reduce_sum(ws[:], es[:], axis=mybir.AxisListType.X)
        nc.scalar.dma_start(wout[:], ws[:])


_NC_CACHE = None


def _get_nc() -> bass.Bass:
    global _NC_CACHE
    if _NC_CACHE is None:
        _NC_CACHE = _build_nc()
    return _NC_CACHE


def _host_prep(audio_embeddings, text_embeddings, semantic_weights, pos_idx):
    """f32 host prep: normalize, positive logits, device operand packing."""
    a = np.asarray(audio_embeddings, dtype=np.float32)
    t = np.asarray(text_embeddings, dtype=np.float32)
    sem = np.asarray(semantic_weights, dtype=np.float32)
    pos = np.asarray(pos_idx, dtype=np.int32)

    an = a / np.maximum(np.linalg.norm(a, axis=1, keepdims=True), 1e-12)
    tn = t / np.maximum(np.linalg.norm(t, axis=1, keepdims=True), 1e-12)
    pos_log = np.einsum("bd,bd->b", an, tn[pos]).astype(np.float32) * np.float32(
        INV_T
    )
    sem_pos = sem[np.arange(B), pos]

    # tT: [D, C] -> [P, NCC, KT, NCHUNK] with k = kt*128 + p, c = cc*512 + j
    t8 = (tn * SF).astype(NP_F8).T.reshape(KT, P, NCC, NCHUNK)
    tt_host = np.ascontiguousarray(t8.transpose(1, 2, 0, 3))

    in_maps = []
    for k in range(NCORES):
        sl = slice(k * BL, (k + 1) * BL)
        # at[p, bt, kt, j] = an[bt*128+j, kt*128+p] * SF
        a8 = (an[sl] * SF).astype(NP_F8).T.reshape(KT, P, NBT, P)
        at_host = np.ascontiguousarray(a8.transpose(1, 2, 0, 3))
        # semc[p, cc, bt, j] = 1 - sem[bt*128+p, cc*512+j]
        s16 = (1.0 - sem[sl]).astype(NP_F8).reshape(NBT, P, NCC, NCHUNK)
        semc_host = np.ascontiguousarray(s16.transpose(1, 2, 0, 3))
        in_maps.append({"at": at_host, "tt": tt_host, "semc": semc_host})
    return in_maps, pos_log, sem_pos


def run_sharded(inputs: dict, trace: bool = False):
    """Run on the 8 NeuronCores; returns (loss_scalar, BassKernelResults)."""
    nc = _get_nc()
    in_maps, pos_log, sem_pos = _host_prep(**inputs)
    res = run_bass_kernel_spmd(
        nc,
        in_maps,
        list(range(NCORES)),
        trace=trace,
        trace_cores=[0] if trace else None,
    )
    # wsum[p, bt] = W[bt*128 + p] for the core's shard
    W = np.concatenate([r["wsum"].T.reshape(BL) for r in res.results])
    den = W + np.exp(pos_log) * sem_pos
    loss = -pos_log + np.log(den)
    val = np.float32(loss.mean(dtype=np.float64))
    return val, res


def kernel(**inputs) -> np.ndarray:
    val, _ = run_sharded(inputs, trace=False)
    return np.asarray(val, dtype=np.float32)
